# revision 2
# baseline (speedup 1.0000x reference)
"""Multi-level ROI Align (FPN pooler, 4 levels summed) on 8 Trainium2 cores.

Strategy: shard ROIs across cores (core k: batch k//4, 128 ROIs). All gather
indices and bilinear weights are computed on host from `boxes`; the device
kernel does the heavy lifting: HBM pixel gathers (dma_gather) + weighted
scatter-reduction into 7x7 bins via PSUM-accumulating matmuls.

Per ROI, per level:
  out[bin, c] = sum_j W[j, bin] * G[j, c]
where G rows are gathered pixel vectors (C=256) and W is sparse (built on
device as fixed_pattern * per-partition scalar for L0/L1, host-baked dense
for the region-gathered L2/L3).

L0 uses 3-pixel elements addressed at even-pixel granularity (idx = flat//2)
to fit the int16 index range (200*200 = 40000 > 32767).
"""
import sys
import numpy as np

sys.path.insert(0, '/opt/trn_rl_repo')

POOLED = 7
SAMP = 2
NBIN = 49
C = 256
IMG = 800.0

# per level: H, W, scale, mode
#   mode 'tri': 3-px elems, idx=flat//2, NJ j's with 3 weight slots
#   mode 'px' : 1-px elems, corner gathers
#   mode 'reg': 1-px elems, bounding-region pixels, host-baked lhsT
L0 = dict(H=200, W=200, scale=0.25, mode='tri', NJ=512, REAL=392, NCH=4)
L1 = dict(H=100, W=100, scale=0.125, mode='px', NJ=896, REAL=784, NCH=7)
L2 = dict(H=50, W=50, scale=0.0625, mode='reg', NJ=384, REAL=324, NCH=3, WREG=18)
L3 = dict(H=25, W=25, scale=0.03125, mode='reg', NJ=128, REAL=100, NCH=1, WREG=10)
LEVELS = [L0, L1, L2, L3]

NROI_CORE = 128     # ROIs per core
NGRP = 64           # groups of 2 ROIs
GRP = 2

# padded flat pixel counts of the feature buffers
F0_ROWS = 40004     # covers 3-px elem overrun
F1_ROWS = 10000
F2_ROWS = 3400      # covers region overrun (y,x up to 66)
F3_ROWS = 900       # covers region overrun (y,x up to 33)

# const fp32 column layout (per partition)
PAT0_OFF = 0                       # [4, 49]
PAT1_OFF = PAT0_OFF + 4 * NBIN     # [7, 49]
WCOL0_OFF = PAT1_OFF + 7 * NBIN    # [128 roi * 12]
WCOL1_OFF = WCOL0_OFF + NROI_CORE * 12   # [128 roi * 7]
ID_OFF = WCOL1_OFF + NROI_CORE * 7       # [49]
CST_COLS = ID_OFF + NBIN

# idx int16 column layout (per partition), per 2-ROI group
IC0, IC1, IC2, IC3 = 64, 112, 48, 16     # cols per group per level
IDX0_OFF = 0
IDX1_OFF = IDX0_OFF + NGRP * IC0
IDX2_OFF = IDX1_OFF + NGRP * IC1
IDX3_OFF = IDX2_OFF + NGRP * IC2
IDX_COLS = IDX3_OFF + NGRP * IC3

_MODULE_CACHE = {}


def _sample_meta(boxes_b, H, W, scale):
    """Per-ROI sample geometry in fp32, matching reference op order.
    boxes_b: [N, 4] fp32. Returns dict of [N,7,2] arrays."""
    f = np.float32
    b = boxes_b.astype(np.float32)
    x1 = b[:, 0] * f(scale)
    y1 = b[:, 1] * f(scale)
    x2 = b[:, 2] * f(scale)
    y2 = b[:, 3] * f(scale)
    rw = np.maximum(x2 - x1, f(1.0))
    rh = np.maximum(y2 - y1, f(1.0))
    bw = rw / f(POOLED)
    bh = rh / f(POOLED)
    g = (np.arange(POOLED, dtype=np.float32)[:, None]
         + (np.arange(SAMP, dtype=np.float32)[None, :] + f(0.5)) / f(SAMP))
    y = y1[:, None, None] + g[None] * bh[:, None, None]   # [N,7,2]
    x = x1[:, None, None] + g[None] * bw[:, None, None]
    masky = ((y >= f(-1.0)) & (y <= f(H))).astype(np.float32)
    maskx = ((x >= f(-1.0)) & (x <= f(W))).astype(np.float32)
    yc = np.clip(y, f(0.0), f(H - 1))
    xc = np.clip(x, f(0.0), f(W - 1))
    yl = np.floor(yc).astype(np.int64)
    xl = np.floor(xc).astype(np.int64)
    yh = np.minimum(yl + 1, H - 1)
    xh = np.minimum(xl + 1, W - 1)
    ly = (yc - yl.astype(np.float32)).astype(np.float32)
    lx = (xc - xl.astype(np.float32)).astype(np.float32)
    hy = (f(1.0) - ly).astype(np.float32)
    hx = (f(1.0) - lx).astype(np.float32)
    return dict(yl=yl, yh=yh, xl=xl, xh=xh, ly=ly, lx=lx, hy=hy, hx=hx,
                masky=masky, maskx=maskx, x=x, y=y)


def _build_tri(meta, lv):
    """L0: j = (row_sel, py, sy, px, sx) -> 392 3-px elems, 3 slot weights.
    Returns idx [N, NJ] int64, w [N, NJ, 3] fp32."""
    N = meta['yl'].shape[0]
    W = lv['W']
    NJ, REAL = lv['NJ'], lv['REAL']
    rows = np.stack([meta['yl'], meta['yh']], axis=1)          # [N,2,7,2] (rs)
    wys = np.stack([meta['hy'], meta['ly']], axis=1)           # [N,2,7,2]
    m = (meta['masky'][:, :, :, None, None] * meta['maskx'][:, None, None, :, :])  # [N,7,2,7,2]
    # broadcast to [N, rs, py, sy, px, sx]
    row = np.broadcast_to(rows[:, :, :, :, None, None], (N, 2, 7, 2, 7, 2))
    wy = np.broadcast_to(wys[:, :, :, :, None, None], (N, 2, 7, 2, 7, 2)).astype(np.float32)
    xl = np.broadcast_to(meta['xl'][:, None, None, None, :, :], (N, 2, 7, 2, 7, 2))
    hx = np.broadcast_to(meta['hx'][:, None, None, None, :, :], (N, 2, 7, 2, 7, 2)).astype(np.float32)
    lx = np.broadcast_to(meta['lx'][:, None, None, None, :, :], (N, 2, 7, 2, 7, 2)).astype(np.float32)
    mm = np.broadcast_to(m[:, None], (N, 2, 7, 2, 7, 2)).astype(np.float32)
    flat = row * W + xl
    idx = (flat >> 1).reshape(N, REAL)
    r = (flat & 1).astype(np.float32).reshape(N, REAL)
    wl = (wy * hx * mm * np.float32(0.25)).reshape(N, REAL)
    wh = (wy * lx * mm * np.float32(0.25)).reshape(N, REAL)
    w = np.zeros((N, NJ, 3), np.float32)
    w[:, :REAL, 0] = wl * (1 - r)
    w[:, :REAL, 1] = wl * r + wh * (1 - r)
    w[:, :REAL, 2] = wh * r
    idx_full = np.zeros((N, NJ), np.int64)
    idx_full[:, :REAL] = idx
    return idx_full, w


def _build_px(meta, lv):
    """L1: j = (row_sel, col_sel, py, sy, px, sx) -> 784 1-px corner gathers.
    Returns idx [N, NJ] int64, w [N, NJ] fp32."""
    N = meta['yl'].shape[0]
    W = lv['W']
    NJ, REAL = lv['NJ'], lv['REAL']
    rows = np.stack([meta['yl'], meta['yh']], axis=1)   # [N,2(rs),7,2]
    wys = np.stack([meta['hy'], meta['ly']], axis=1)
    cols = np.stack([meta['xl'], meta['xh']], axis=1)   # [N,2(cs),7,2]
    wxs = np.stack([meta['hx'], meta['lx']], axis=1)
    m = (meta['masky'][:, :, :, None, None] * meta['maskx'][:, None, None, :, :])
    row = np.broadcast_to(rows[:, :, None, :, :, None, None], (N, 2, 2, 7, 2, 7, 2))
    wy = np.broadcast_to(wys[:, :, None, :, :, None, None], (N, 2, 2, 7, 2, 7, 2)).astype(np.float32)
    col = np.broadcast_to(cols[:, None, :, None, None, :, :], (N, 2, 2, 7, 2, 7, 2))
    wx = np.broadcast_to(wxs[:, None, :, None, None, :, :], (N, 2, 2, 7, 2, 7, 2)).astype(np.float32)
    mm = np.broadcast_to(m[:, None, None], (N, 2, 2, 7, 2, 7, 2)).astype(np.float32)
    idx = (row * W + col).reshape(N, REAL)
    w = (wy * wx * mm * np.float32(0.25)).reshape(N, REAL)
    idx_full = np.zeros((N, NJ), np.int64)
    w_full = np.zeros((N, NJ), np.float32)
    idx_full[:, :REAL] = idx
    w_full[:, :REAL] = w
    return idx_full, w_full


def _build_reg(meta, lv):
    """L2/L3: bounding-region pixels + separable host-baked weights.
    Returns idx [N, NJ] int64, lhsT [N, NJ, 49] fp32."""
    N = meta['yl'].shape[0]
    H, W, WREG = lv['H'], lv['W'], lv['WREG']
    NJ, REAL = lv['NJ'], lv['REAL']
    f = np.float32
    y_base = np.floor(np.clip(meta['y'].reshape(N, -1).min(1), 0.0, H - 1)).astype(np.int64)
    x_base = np.floor(np.clip(meta['x'].reshape(N, -1).min(1), 0.0, W - 1)).astype(np.int64)
    # WY [N, WREG, 7], WX [N, WREG, 7]
    WY = np.zeros((N, WREG, POOLED), np.float32)
    WX = np.zeros((N, WREG, POOLED), np.float32)
    ridx = np.arange(N)[:, None, None]
    pidx = np.broadcast_to(np.arange(POOLED)[None, :, None], (N, POOLED, SAMP))
    np.add.at(WY, (ridx, meta['yl'] - y_base[:, None, None], pidx),
              (f(0.5) * meta['hy'] * meta['masky']).astype(np.float32))
    np.add.at(WY, (ridx, meta['yh'] - y_base[:, None, None], pidx),
              (f(0.5) * meta['ly'] * meta['masky']).astype(np.float32))
    np.add.at(WX, (ridx, meta['xl'] - x_base[:, None, None], pidx),
              (f(0.5) * meta['hx'] * meta['maskx']).astype(np.float32))
    np.add.at(WX, (ridx, meta['xh'] - x_base[:, None, None], pidx),
              (f(0.5) * meta['lx'] * meta['maskx']).astype(np.float32))
    lhsT = np.einsum('nap,nbq->nabpq', WY, WX).reshape(N, REAL, NBIN)
    dy = np.arange(WREG)
    idx = ((y_base[:, None, None] + dy[None, :, None]) * W
           + x_base[:, None, None] + dy[None, None, :]).reshape(N, REAL)
    idx_full = np.zeros((N, NJ), np.int64)
    lhsT_full = np.zeros((N, NJ, NBIN), np.float32)
    idx_full[:, :REAL] = idx
    lhsT_full[:, :REAL] = lhsT
    return idx_full, lhsT_full


def _pack_idx(jlists):
    """Pack concatenated per-group idx list [NJ_total] -> [128, NJ_total//16]
    int16 wrapped in 16 partitions, replicated 8x."""
    jl = np.asarray(jlists)
    n = jl.shape[-1]
    arr = jl.reshape(*jl.shape[:-1], n // 16, 16)   # [..., col, p]
    arr = np.swapaxes(arr, -1, -2)                  # [..., p(16), col]
    arr = np.broadcast_to(arr[..., None, :, :],
                          (*jl.shape[:-1], 8, 16, n // 16))
    return arr.reshape(*jl.shape[:-1], 128, n // 16).astype(np.int16)


def _bin_pattern(mode, NCH, REAL):
    """Fixed j->bin one-hot pattern [128, NCH, 49] for 'tri'/'px' j order."""
    NJ = NCH * 128
    j = np.arange(NJ)
    if mode == 'tri':
        # j = ((((rs*7+py)*2+sy)*7+px)*2+sx)
        px = (j // 2) % 7
        py = (j // (2 * 7 * 2)) % 7
    else:
        # j = (((((rs*2+cs)*7+py)*2+sy)*7+px)*2+sx)
        px = (j // 2) % 7
        py = (j // (2 * 7 * 2)) % 7
    bins = py * 7 + px
    pat = np.zeros((NJ, NBIN), np.float32)
    valid = j < REAL
    pat[np.arange(NJ)[valid], bins[valid]] = 1.0
    return pat.reshape(NCH, 128, NBIN).transpose(1, 0, 2)   # [128, NCH, 49]


def _host_prepare(x0, x1, x2, x3, boxes):
    """Build all per-core input tensors. Returns list of 8 dicts."""
    B = boxes.shape[0]
    feats = []
    for arr, lv, rows in ((x0, L0, F0_ROWS), (x1, L1, F1_ROWS),
                          (x2, L2, F2_ROWS), (x3, L3, F3_ROWS)):
        f = np.zeros((B, rows, C), np.float32)
        hw = lv['H'] * lv['W']
        f[:, :hw] = np.ascontiguousarray(
            np.transpose(np.asarray(arr, np.float32), (0, 2, 3, 1))).reshape(B, hw, C)
        feats.append(f)

    per_batch = []
    for b in range(B):
        bb = np.asarray(boxes[b], np.float32)
        m0 = _sample_meta(bb, L0['H'], L0['W'], L0['scale'])
        m1 = _sample_meta(bb, L1['H'], L1['W'], L1['scale'])
        m2 = _sample_meta(bb, L2['H'], L2['W'], L2['scale'])
        m3 = _sample_meta(bb, L3['H'], L3['W'], L3['scale'])
        idx0, w0 = _build_tri(m0, L0)
        idx1, w1 = _build_px(m1, L1)
        idx2, lt2 = _build_reg(m2, L2)
        idx3, lt3 = _build_reg(m3, L3)
        per_batch.append((idx0, w0, idx1, w1, idx2, lt2, idx3, lt3))

    pat0 = _bin_pattern('tri', L0['NCH'], L0['REAL'])
    pat1 = _bin_pattern('px', L1['NCH'], L1['REAL'])

    in_maps = []
    for k in range(8):
        b = k // 4
        s = (k % 4) * NROI_CORE
        idx0, w0, idx1, w1, idx2, lt2, idx3, lt3 = per_batch[b]
        sl = slice(s, s + NROI_CORE)

        cst = np.zeros((128, CST_COLS), np.float32)
        cst[:, PAT0_OFF:PAT0_OFF + 4 * NBIN] = pat0.reshape(128, -1)
        cst[:, PAT1_OFF:PAT1_OFF + 7 * NBIN] = pat1.reshape(128, -1)
        # wcol0 [128, roi*12]: col roi*12 + c*3 + slot = w0[roi, c*128+p, slot]
        wc0 = w0[sl].reshape(NROI_CORE, L0['NCH'], 128, 3)   # [roi,c,p,s]
        cst[:, WCOL0_OFF:WCOL0_OFF + NROI_CORE * 12] = (
            wc0.transpose(2, 0, 1, 3).reshape(128, -1))
        wc1 = w1[sl].reshape(NROI_CORE, L1['NCH'], 128)      # [roi,c,p]
        cst[:, WCOL1_OFF:WCOL1_OFF + NROI_CORE * 7] = (
            wc1.transpose(2, 0, 1).reshape(128, -1))
        cst[:NBIN, ID_OFF:ID_OFF + NBIN] = np.eye(NBIN, dtype=np.float32)

        idxs = np.zeros((128, IDX_COLS), np.int16)
        idxs[:, IDX0_OFF:IDX0_OFF + NGRP * IC0] = _pack_idx(
            idx0[sl].reshape(NGRP, GRP * L0['NJ'])).transpose(1, 0, 2).reshape(128, -1)
        idxs[:, IDX1_OFF:IDX1_OFF + NGRP * IC1] = _pack_idx(
            idx1[sl].reshape(NGRP, GRP * L1['NJ'])).transpose(1, 0, 2).reshape(128, -1)
        idxs[:, IDX2_OFF:IDX2_OFF + NGRP * IC2] = _pack_idx(
            idx2[sl].reshape(NGRP, GRP * L2['NJ'])).transpose(1, 0, 2).reshape(128, -1)
        idxs[:, IDX3_OFF:IDX3_OFF + NGRP * IC3] = _pack_idx(
            idx3[sl].reshape(NGRP, GRP * L3['NJ'])).transpose(1, 0, 2).reshape(128, -1)

        # lhsT k-major: lt2 [roi, NJ(=3*128), 49] -> [roi, 128, 3, 49]
        lt2k = np.ascontiguousarray(
            lt2[sl].reshape(NROI_CORE, L2['NCH'], 128, NBIN).transpose(0, 2, 1, 3))
        lt3k = np.ascontiguousarray(lt3[sl].reshape(NROI_CORE, 128, NBIN))

        in_maps.append({
            "f0": feats[0][b], "f1": feats[1][b],
            "f2": feats[2][b], "f3": feats[3][b],
            "cst": cst, "idxs": idxs, "lt2": lt2k, "lt3": lt3k,
        })
    return in_maps


def _build_module():
    from concourse import bacc, tile
    from concourse.bass import mybir
    import concourse.bass as bass_mod

    F32 = mybir.dt.float32
    I16 = mybir.dt.int16
    AP = bass_mod.AP

    nc = bacc.Bacc(None, target_bir_lowering=False)
    f0 = nc.dram_tensor("f0", [F0_ROWS, C], F32, kind="ExternalInput")
    f1 = nc.dram_tensor("f1", [F1_ROWS, C], F32, kind="ExternalInput")
    f2 = nc.dram_tensor("f2", [F2_ROWS, C], F32, kind="ExternalInput")
    f3 = nc.dram_tensor("f3", [F3_ROWS, C], F32, kind="ExternalInput")
    cst = nc.dram_tensor("cst", [128, CST_COLS], F32, kind="ExternalInput")
    idxs = nc.dram_tensor("idxs", [128, IDX_COLS], I16, kind="ExternalInput")
    lt2 = nc.dram_tensor("lt2", [NROI_CORE, 128, L2['NCH'], NBIN], F32, kind="ExternalInput")
    lt3 = nc.dram_tensor("lt3", [NROI_CORE, 128, NBIN], F32, kind="ExternalInput")
    out = nc.dram_tensor("out", [NROI_CORE, C, NBIN], F32, kind="ExternalOutput")

    # overlapping 3-px elem view of f0: stride 2px, width 3px
    f0_view = AP(f0, 0, [[2 * C, F0_ROWS // 2 - 1], [1, 3 * C]])
    gather_srcs = [f0_view, f1[:], f2[:], f3[:]]
    ELEM = [3 * C, C, C, C]
    STEP = [2 * C, C, C, C]
    ICOLS = [IC0, IC1, IC2, IC3]
    IOFF = [IDX0_OFF, IDX1_OFF, IDX2_OFF, IDX3_OFF]

    with tile.TileContext(nc) as tc:
        with (
            tc.tile_pool(name="const", bufs=1) as constp,
            tc.tile_pool(name="g0p", bufs=2) as g0p,
            tc.tile_pool(name="g1p", bufs=2) as g1p,
            tc.tile_pool(name="g2p", bufs=2) as g2p,
            tc.tile_pool(name="g3p", bufs=2) as g3p,
            tc.tile_pool(name="ltp", bufs=3) as ltp,
            tc.tile_pool(name="wp", bufs=6) as wp,
            tc.tile_pool(name="accp", bufs=4, space="PSUM") as accp,
            tc.tile_pool(name="ptp", bufs=2, space="PSUM") as ptp,
            tc.tile_pool(name="evp", bufs=3) as evp,
            tc.tile_pool(name="otp", bufs=3) as otp,
        ):
            cst_t = constp.tile([128, CST_COLS], F32)
            nc.sync.dma_start(cst_t[:], cst[:])
            idx_t = constp.tile([128, IDX_COLS], I16)
            nc.sync.dma_start(idx_t[:], idxs[:])

            gpools = [g0p, g1p, g2p, g3p]
            for grp in range(NGRP):
                gts = []
                for l, lv in enumerate(LEVELS):
                    nidx = GRP * lv['NJ']
                    gt = gpools[l].tile([128, GRP * lv['NCH'], ELEM[l]], F32,
                                        tag=f"g{l}")
                    io = IOFF[l] + grp * ICOLS[l]
                    if nidx <= 1024:
                        nc.gpsimd.dma_gather(
                            gt[:], gather_srcs[l], idx_t[:, io:io + ICOLS[l]],
                            nidx, nidx, ELEM[l], elem_step=STEP[l])
                    else:
                        # SWDGE ring cap: split into one call per ROI
                        hc = ICOLS[l] // GRP
                        for r2 in range(GRP):
                            nc.gpsimd.dma_gather(
                                gt[:, r2 * lv['NCH']:(r2 + 1) * lv['NCH'], :],
                                gather_srcs[l],
                                idx_t[:, io + r2 * hc:io + (r2 + 1) * hc],
                                lv['NJ'], lv['NJ'], ELEM[l], elem_step=STEP[l])
                    gts.append(gt)

                for r2 in range(GRP):
                    roi = grp * GRP + r2
                    lt2_t = ltp.tile([128, L2['NCH'], NBIN], F32, tag="lt2")
                    nc.sync.dma_start(lt2_t[:], lt2[roi])
                    lt3_t = ltp.tile([128, NBIN], F32, tag="lt3")
                    nc.sync.dma_start(lt3_t[:], lt3[roi])

                    acc = accp.tile([NBIN, C], F32)
                    n_mm = 12 + 7 + 3 + 1
                    mi = 0
                    # L0: 4 chunks x 3 slots
                    for c in range(L0['NCH']):
                        for s in range(3):
                            w = wp.tile([128, NBIN], F32, tag="w")
                            colw = WCOL0_OFF + roi * 12 + c * 3 + s
                            nc.vector.tensor_scalar_mul(
                                w[:],
                                cst_t[:, PAT0_OFF + c * NBIN:PAT0_OFF + (c + 1) * NBIN],
                                cst_t[:, colw:colw + 1])
                            nc.tensor.matmul(
                                acc[:], w[:],
                                gts[0][:, r2 * L0['NCH'] + c, s * C:(s + 1) * C],
                                start=(mi == 0), stop=(mi == n_mm - 1))
                            mi += 1
                    # L1: 7 chunks
                    for c in range(L1['NCH']):
                        w = wp.tile([128, NBIN], F32, tag="w")
                        colw = WCOL1_OFF + roi * 7 + c
                        nc.vector.tensor_scalar_mul(
                            w[:],
                            cst_t[:, PAT1_OFF + c * NBIN:PAT1_OFF + (c + 1) * NBIN],
                            cst_t[:, colw:colw + 1])
                        nc.tensor.matmul(
                            acc[:], w[:], gts[1][:, r2 * L1['NCH'] + c, :],
                            start=(mi == 0), stop=(mi == n_mm - 1))
                        mi += 1
                    # L2: 3 chunks, host-baked lhsT
                    for c in range(L2['NCH']):
                        nc.tensor.matmul(
                            acc[:], lt2_t[:, c, :], gts[2][:, r2 * L2['NCH'] + c, :],
                            start=(mi == 0), stop=(mi == n_mm - 1))
                        mi += 1
                    # L3: 1 chunk
                    nc.tensor.matmul(
                        acc[:], lt3_t[:], gts[3][:, r2, :],
                        start=(mi == 0), stop=(mi == n_mm - 1))
                    mi += 1

                    ev = evp.tile([NBIN, C], F32, tag="ev")
                    nc.scalar.copy(ev[:], acc[:])
                    pt = ptp.tile([128, 2, NBIN], F32, tag="pt")
                    for h in range(2):
                        nc.tensor.transpose(
                            pt[:, h, :], ev[:, h * 128:(h + 1) * 128],
                            cst_t[:NBIN, ID_OFF:ID_OFF + NBIN])
                    ot = otp.tile([128, 2, NBIN], F32, tag="ot")
                    nc.vector.tensor_copy(ot[:], pt[:])
                    # out[roi] is [256, 49]; view as [h, p, m] -> dst [p, h, m]
                    dst = out[roi].rearrange("(h p) m -> p h m", h=2)
                    nc.sync.dma_start(dst, ot[:])
    nc.finalize()
    return nc


def kernel(x0, x1, x2, x3, boxes):
    from concourse.bass_utils import run_bass_kernel_spmd
    in_maps = _host_prepare(x0, x1, x2, x3, boxes)
    if 'nc' not in _MODULE_CACHE:
        _MODULE_CACHE['nc'] = _build_module()
    nc = _MODULE_CACHE['nc']
    res = run_bass_kernel_spmd(nc, in_maps, list(range(8)))
    globals()['_LAST_RESULTS'] = res
    outs = [res.results[k]["out"] for k in range(8)]
    full = np.concatenate(outs, axis=0)           # [1024, 256, 49]
    return full.reshape(1024, C, POOLED, POOLED).astype(np.float32)



# revision 6
# speedup vs baseline: 1.1008x; 1.1008x over previous
"""Multi-level ROI Align (FPN pooler, 4 levels summed) on 8 Trainium2 cores.

Strategy: shard ROIs across cores (core k: batch k//4, 128 ROIs). Gather
indices and bilinear weights are computed on host from `boxes`; the device
kernel does HBM pixel gathers (bf16) + weighted scatter-reduction into 7x7
bins via PSUM-accumulating bf16 matmuls.

Per ROI, per level:  out[bin, c] = sum_j W[j, bin] * G[j, c]
where G rows are gathered bf16 pixel vectors (C=256) and W is either
fixed_pattern * per-partition scalar built on DVE (L0/L1, one-hot j->bin)
or host-baked dense bf16 lhsT (L2/L3 region mode).

L0 uses 3-px elements addressed at even-pixel granularity (idx = flat//2)
to fit the int16 index range (200*200 = 40000 > 32767).

Output is accumulated in fp32 PSUM, evacuated as bf16 [49, C] per ROI and
DMA'd straight to HBM; the host does the final [49,C] -> [C,7,7] transpose.
"""
import sys
import numpy as np
import ml_dtypes

sys.path.insert(0, '/opt/trn_rl_repo')

BF16 = ml_dtypes.bfloat16

POOLED = 7
SAMP = 2
NBIN = 49
C = 256
IMG = 800.0

# per level: H, W, scale, mode ('tri' 3px elems idx=flat//2 | 'px' 1px | 'reg' region px)
L0 = dict(H=200, W=200, scale=0.25, mode='tri', NJ=512, REAL=392, NCH=4)
L1 = dict(H=100, W=100, scale=0.125, mode='px', NJ=896, REAL=784, NCH=7)
L2 = dict(H=50, W=50, scale=0.0625, mode='reg', NJ=384, REAL=324, NCH=3, WREG=18)
L3 = dict(H=25, W=25, scale=0.03125, mode='reg', NJ=128, REAL=100, NCH=1, WREG=10)
LEVELS = [L0, L1, L2, L3]

NROI_CORE = 128     # ROIs per core
NGRP = 64           # groups of 2 ROIs
GRP = 2
BLK2 = 2            # ROIs per L2 gather call (HW SWDGE ring caps a call at 1024 descs)
BLK3 = 8            # ROIs per L3 gather call

# padded flat pixel counts of the feature buffers
F0_ROWS = 40004     # covers 3-px elem overrun
F1_ROWS = 10000
F2_ROWS = 3400      # covers region overrun (y,x up to 66)
F3_ROWS = 900       # covers region overrun (y,x up to 33)

# const bf16 column layout (per partition)
PAT0_OFF = 0                                  # [12, 49] pattern expanded per slot
PAT1_OFF = PAT0_OFF + 12 * NBIN               # [7, 49]
WCOL0_OFF = PAT1_OFF + 7 * NBIN               # [128 roi * 12]
WCOL1_OFF = WCOL0_OFF + NROI_CORE * 12        # [128 roi * 7]
CST_COLS = WCOL1_OFF + NROI_CORE * 7

# idx int16 column layout (per partition)
IC0 = GRP * L0['NJ'] // 16                    # 64 cols per 2-ROI group
IC1 = GRP * L1['NJ'] // 16                    # 112
IC2 = BLK2 * L2['NJ'] // 16                   # 192 cols per 8-ROI block
IC3 = BLK3 * L3['NJ'] // 16                   # 128 cols per 16-ROI block
IDX0_OFF = 0
IDX1_OFF = IDX0_OFF + NGRP * IC0
IDX2_OFF = IDX1_OFF + NGRP * IC1
IDX3_OFF = IDX2_OFF + (NROI_CORE // BLK2) * IC2
IDX_COLS = IDX3_OFF + (NROI_CORE // BLK3) * IC3

_MODULE_CACHE = {}


def _sample_meta(boxes_b, H, W, scale):
    """Per-ROI sample geometry in fp32, matching reference op order.
    boxes_b: [N, 4] fp32. Returns dict of [N,7,2] arrays."""
    f = np.float32
    b = boxes_b.astype(np.float32)
    x1 = b[:, 0] * f(scale)
    y1 = b[:, 1] * f(scale)
    x2 = b[:, 2] * f(scale)
    y2 = b[:, 3] * f(scale)
    rw = np.maximum(x2 - x1, f(1.0))
    rh = np.maximum(y2 - y1, f(1.0))
    bw = rw / f(POOLED)
    bh = rh / f(POOLED)
    g = (np.arange(POOLED, dtype=np.float32)[:, None]
         + (np.arange(SAMP, dtype=np.float32)[None, :] + f(0.5)) / f(SAMP))
    y = y1[:, None, None] + g[None] * bh[:, None, None]   # [N,7,2]
    x = x1[:, None, None] + g[None] * bw[:, None, None]
    masky = ((y >= f(-1.0)) & (y <= f(H))).astype(np.float32)
    maskx = ((x >= f(-1.0)) & (x <= f(W))).astype(np.float32)
    yc = np.clip(y, f(0.0), f(H - 1))
    xc = np.clip(x, f(0.0), f(W - 1))
    yl = np.floor(yc).astype(np.int64)
    xl = np.floor(xc).astype(np.int64)
    yh = np.minimum(yl + 1, H - 1)
    xh = np.minimum(xl + 1, W - 1)
    ly = (yc - yl.astype(np.float32)).astype(np.float32)
    lx = (xc - xl.astype(np.float32)).astype(np.float32)
    hy = (f(1.0) - ly).astype(np.float32)
    hx = (f(1.0) - lx).astype(np.float32)
    return dict(yl=yl, yh=yh, xl=xl, xh=xh, ly=ly, lx=lx, hy=hy, hx=hx,
                masky=masky, maskx=maskx, x=x, y=y)


def _build_tri(meta, lv):
    """L0: j = (rs, py, sy, px, sx) -> 392 3-px elems, 3 slot weights.
    Returns idx [N, NJ] int64, w [N, NJ, 3] fp32."""
    N = meta['yl'].shape[0]
    W = lv['W']
    NJ, REAL = lv['NJ'], lv['REAL']
    rows = np.stack([meta['yl'], meta['yh']], axis=1)          # [N,2,7,2] (rs)
    wys = np.stack([meta['hy'], meta['ly']], axis=1)           # [N,2,7,2]
    m = (meta['masky'][:, :, :, None, None] * meta['maskx'][:, None, None, :, :])  # [N,7,2,7,2]
    row = np.broadcast_to(rows[:, :, :, :, None, None], (N, 2, 7, 2, 7, 2))
    wy = np.broadcast_to(wys[:, :, :, :, None, None], (N, 2, 7, 2, 7, 2)).astype(np.float32)
    xl = np.broadcast_to(meta['xl'][:, None, None, None, :, :], (N, 2, 7, 2, 7, 2))
    hx = np.broadcast_to(meta['hx'][:, None, None, None, :, :], (N, 2, 7, 2, 7, 2)).astype(np.float32)
    lx = np.broadcast_to(meta['lx'][:, None, None, None, :, :], (N, 2, 7, 2, 7, 2)).astype(np.float32)
    mm = np.broadcast_to(m[:, None], (N, 2, 7, 2, 7, 2)).astype(np.float32)
    flat = row * W + xl
    idx = (flat >> 1).reshape(N, REAL)
    r = (flat & 1).astype(np.float32).reshape(N, REAL)
    wl = (wy * hx * mm * np.float32(0.25)).reshape(N, REAL)
    wh = (wy * lx * mm * np.float32(0.25)).reshape(N, REAL)
    w = np.zeros((N, NJ, 3), np.float32)
    w[:, :REAL, 0] = wl * (1 - r)
    w[:, :REAL, 1] = wl * r + wh * (1 - r)
    w[:, :REAL, 2] = wh * r
    idx_full = np.zeros((N, NJ), np.int64)
    idx_full[:, :REAL] = idx
    return idx_full, w


def _build_px(meta, lv):
    """L1: j = (rs, cs, py, sy, px, sx) -> 784 1-px corner gathers.
    Returns idx [N, NJ] int64, w [N, NJ] fp32."""
    N = meta['yl'].shape[0]
    W = lv['W']
    NJ, REAL = lv['NJ'], lv['REAL']
    rows = np.stack([meta['yl'], meta['yh']], axis=1)   # [N,2(rs),7,2]
    wys = np.stack([meta['hy'], meta['ly']], axis=1)
    cols = np.stack([meta['xl'], meta['xh']], axis=1)   # [N,2(cs),7,2]
    wxs = np.stack([meta['hx'], meta['lx']], axis=1)
    m = (meta['masky'][:, :, :, None, None] * meta['maskx'][:, None, None, :, :])
    row = np.broadcast_to(rows[:, :, None, :, :, None, None], (N, 2, 2, 7, 2, 7, 2))
    wy = np.broadcast_to(wys[:, :, None, :, :, None, None], (N, 2, 2, 7, 2, 7, 2)).astype(np.float32)
    col = np.broadcast_to(cols[:, None, :, None, None, :, :], (N, 2, 2, 7, 2, 7, 2))
    wx = np.broadcast_to(wxs[:, None, :, None, None, :, :], (N, 2, 2, 7, 2, 7, 2)).astype(np.float32)
    mm = np.broadcast_to(m[:, None, None], (N, 2, 2, 7, 2, 7, 2)).astype(np.float32)
    idx = (row * W + col).reshape(N, REAL)
    w = (wy * wx * mm * np.float32(0.25)).reshape(N, REAL)
    idx_full = np.zeros((N, NJ), np.int64)
    w_full = np.zeros((N, NJ), np.float32)
    idx_full[:, :REAL] = idx
    w_full[:, :REAL] = w
    return idx_full, w_full


def _build_reg(meta, lv):
    """L2/L3: bounding-region pixels + separable host-baked weights.
    Returns idx [N, NJ] int64, lhsT [N, NJ, 49] fp32."""
    N = meta['yl'].shape[0]
    H, W, WREG = lv['H'], lv['W'], lv['WREG']
    NJ, REAL = lv['NJ'], lv['REAL']
    f = np.float32
    y_base = np.floor(np.clip(meta['y'].reshape(N, -1).min(1), 0.0, H - 1)).astype(np.int64)
    x_base = np.floor(np.clip(meta['x'].reshape(N, -1).min(1), 0.0, W - 1)).astype(np.int64)
    WY = np.zeros((N, WREG, POOLED), np.float32)
    WX = np.zeros((N, WREG, POOLED), np.float32)
    ridx = np.arange(N)[:, None, None]
    pidx = np.broadcast_to(np.arange(POOLED)[None, :, None], (N, POOLED, SAMP))
    np.add.at(WY, (ridx, meta['yl'] - y_base[:, None, None], pidx),
              (f(0.5) * meta['hy'] * meta['masky']).astype(np.float32))
    np.add.at(WY, (ridx, meta['yh'] - y_base[:, None, None], pidx),
              (f(0.5) * meta['ly'] * meta['masky']).astype(np.float32))
    np.add.at(WX, (ridx, meta['xl'] - x_base[:, None, None], pidx),
              (f(0.5) * meta['hx'] * meta['maskx']).astype(np.float32))
    np.add.at(WX, (ridx, meta['xh'] - x_base[:, None, None], pidx),
              (f(0.5) * meta['lx'] * meta['maskx']).astype(np.float32))
    lhsT = np.einsum('nap,nbq->nabpq', WY, WX).reshape(N, REAL, NBIN)
    dy = np.arange(WREG)
    idx = ((y_base[:, None, None] + dy[None, :, None]) * W
           + x_base[:, None, None] + dy[None, None, :]).reshape(N, REAL)
    idx_full = np.zeros((N, NJ), np.int64)
    lhsT_full = np.zeros((N, NJ, NBIN), np.float32)
    idx_full[:, :REAL] = idx
    lhsT_full[:, :REAL] = lhsT
    return idx_full, lhsT_full


def _pack_idx(jlists):
    """Pack concatenated idx list [..., n] -> [..., 128, n//16]
    int16 wrapped in 16 partitions, replicated 8x."""
    jl = np.asarray(jlists)
    n = jl.shape[-1]
    arr = jl.reshape(*jl.shape[:-1], n // 16, 16)   # [..., col, p]
    arr = np.swapaxes(arr, -1, -2)                  # [..., p(16), col]
    arr = np.broadcast_to(arr[..., None, :, :],
                          (*jl.shape[:-1], 8, 16, n // 16))
    return arr.reshape(*jl.shape[:-1], 128, n // 16).astype(np.int16)


def _bin_pattern(mode, NCH, REAL, nslot):
    """Fixed j->bin one-hot pattern [128, NCH*nslot, 49] (expanded per slot)."""
    NJ = NCH * 128
    j = np.arange(NJ)
    # j = ((((rs*7+py)*2+sy)*7+px)*2+sx)  (same py/px decode for tri & px)
    px = (j // 2) % 7
    py = (j // (2 * 7 * 2)) % 7
    bins = py * 7 + px
    pat = np.zeros((NJ, NBIN), np.float32)
    valid = j < REAL
    pat[np.arange(NJ)[valid], bins[valid]] = 1.0
    pat = pat.reshape(NCH, 128, NBIN).transpose(1, 0, 2)          # [128, NCH, 49]
    pat = np.repeat(pat[:, :, None, :], nslot, axis=2)            # [128, NCH, nslot, 49]
    return pat.reshape(128, NCH * nslot, NBIN)


def _host_prepare(x0, x1, x2, x3, boxes):
    """Build all per-core input tensors. Returns list of 8 dicts."""
    B = boxes.shape[0]
    feats = []
    for arr, lv, rows in ((x0, L0, F0_ROWS), (x1, L1, F1_ROWS),
                          (x2, L2, F2_ROWS), (x3, L3, F3_ROWS)):
        f = np.zeros((B, rows, C), BF16)
        hw = lv['H'] * lv['W']
        f[:, :hw] = np.ascontiguousarray(
            np.transpose(np.asarray(arr, np.float32), (0, 2, 3, 1))).reshape(B, hw, C).astype(BF16)
        feats.append(f)

    per_batch = []
    for b in range(B):
        bb = np.asarray(boxes[b], np.float32)
        m0 = _sample_meta(bb, L0['H'], L0['W'], L0['scale'])
        m1 = _sample_meta(bb, L1['H'], L1['W'], L1['scale'])
        m2 = _sample_meta(bb, L2['H'], L2['W'], L2['scale'])
        m3 = _sample_meta(bb, L3['H'], L3['W'], L3['scale'])
        idx0, w0 = _build_tri(m0, L0)
        idx1, w1 = _build_px(m1, L1)
        idx2, lt2 = _build_reg(m2, L2)
        idx3, lt3 = _build_reg(m3, L3)
        per_batch.append((idx0, w0, idx1, w1, idx2, lt2, idx3, lt3))

    pat0 = _bin_pattern('tri', L0['NCH'], L0['REAL'], 3)   # [128, 12, 49]
    pat1 = _bin_pattern('px', L1['NCH'], L1['REAL'], 1)    # [128, 7, 49]

    in_maps = []
    for k in range(8):
        b = k // 4
        s = (k % 4) * NROI_CORE
        idx0, w0, idx1, w1, idx2, lt2, idx3, lt3 = per_batch[b]
        sl = slice(s, s + NROI_CORE)

        cst = np.zeros((128, CST_COLS), BF16)
        cst[:, PAT0_OFF:PAT0_OFF + 12 * NBIN] = pat0.reshape(128, -1).astype(BF16)
        cst[:, PAT1_OFF:PAT1_OFF + 7 * NBIN] = pat1.reshape(128, -1).astype(BF16)
        # wcol0 [128, roi*12]: col roi*12 + c*3 + slot = w0[roi, c*128+p, slot]
        wc0 = w0[sl].reshape(NROI_CORE, L0['NCH'], 128, 3)   # [roi,c,p,s]
        cst[:, WCOL0_OFF:WCOL0_OFF + NROI_CORE * 12] = (
            wc0.transpose(2, 0, 1, 3).reshape(128, -1).astype(BF16))
        wc1 = w1[sl].reshape(NROI_CORE, L1['NCH'], 128)      # [roi,c,p]
        cst[:, WCOL1_OFF:WCOL1_OFF + NROI_CORE * 7] = (
            wc1.transpose(2, 0, 1).reshape(128, -1).astype(BF16))

        idxs = np.zeros((128, IDX_COLS), np.int16)
        idxs[:, IDX0_OFF:IDX0_OFF + NGRP * IC0] = _pack_idx(
            idx0[sl].reshape(NGRP, GRP * L0['NJ'])).transpose(1, 0, 2).reshape(128, -1)
        idxs[:, IDX1_OFF:IDX1_OFF + NGRP * IC1] = _pack_idx(
            idx1[sl].reshape(NGRP, GRP * L1['NJ'])).transpose(1, 0, 2).reshape(128, -1)
        idxs[:, IDX2_OFF:IDX2_OFF + (NROI_CORE // BLK2) * IC2] = _pack_idx(
            idx2[sl].reshape(NROI_CORE // BLK2, BLK2 * L2['NJ'])).transpose(1, 0, 2).reshape(128, -1)
        idxs[:, IDX3_OFF:IDX3_OFF + (NROI_CORE // BLK3) * IC3] = _pack_idx(
            idx3[sl].reshape(NROI_CORE // BLK3, BLK3 * L3['NJ'])).transpose(1, 0, 2).reshape(128, -1)

        # lhsT k-major bf16: lt2 [roi, NJ(=3*128), 49] -> [roi, 128, 3*49]
        lt2k = np.ascontiguousarray(
            lt2[sl].reshape(NROI_CORE, L2['NCH'], 128, NBIN).transpose(0, 2, 1, 3)
        ).reshape(NROI_CORE, 128, L2['NCH'] * NBIN).astype(BF16)
        lt3k = np.ascontiguousarray(lt3[sl].reshape(NROI_CORE, 128, NBIN)).astype(BF16)

        in_maps.append({
            "f0": feats[0][b], "f1": feats[1][b],
            "f2": feats[2][b], "f3": feats[3][b],
            "cst": cst, "idxs": idxs, "lt2": lt2k, "lt3": lt3k,
        })
    return in_maps


def _build_module():
    from concourse import bacc, tile
    from concourse.bass import mybir
    import concourse.bass as bass_mod

    F32 = mybir.dt.float32
    BF = mybir.dt.bfloat16
    I16 = mybir.dt.int16
    AP = bass_mod.AP

    nc = bacc.Bacc(None, target_bir_lowering=False)
    f0 = nc.dram_tensor("f0", [F0_ROWS, C], BF, kind="ExternalInput")
    f1 = nc.dram_tensor("f1", [F1_ROWS, C], BF, kind="ExternalInput")
    f2 = nc.dram_tensor("f2", [F2_ROWS, C], BF, kind="ExternalInput")
    f3 = nc.dram_tensor("f3", [F3_ROWS, C], BF, kind="ExternalInput")
    cst = nc.dram_tensor("cst", [128, CST_COLS], BF, kind="ExternalInput")
    idxs = nc.dram_tensor("idxs", [128, IDX_COLS], I16, kind="ExternalInput")
    lt2 = nc.dram_tensor("lt2", [NROI_CORE, 128, L2['NCH'] * NBIN], BF, kind="ExternalInput")
    lt3 = nc.dram_tensor("lt3", [NROI_CORE, 128, NBIN], BF, kind="ExternalInput")
    out = nc.dram_tensor("out", [NROI_CORE, NBIN, C], BF, kind="ExternalOutput")

    # overlapping 3-px elem view of f0: stride 2px, width 3px
    f0_view = AP(f0, 0, [[2 * C, F0_ROWS // 2 - 1], [1, 3 * C]])

    with tile.TileContext(nc) as tc:
        with (
            tc.tile_pool(name="const", bufs=1) as constp,
            tc.tile_pool(name="g0p", bufs=2) as g0p,
            tc.tile_pool(name="g1p", bufs=2) as g1p,
            tc.tile_pool(name="g2p", bufs=2) as g2p,
            tc.tile_pool(name="g3p", bufs=2) as g3p,
            tc.tile_pool(name="ltp", bufs=2) as ltp,
            tc.tile_pool(name="wp", bufs=4) as wp,
            tc.tile_pool(name="accp", bufs=4, space="PSUM") as accp,
            tc.tile_pool(name="evp", bufs=3) as evp,
        ):
            cst_t = constp.tile([128, CST_COLS], BF)
            nc.sync.dma_start(cst_t[:], cst[:])
            idx_t = constp.tile([128, IDX_COLS], I16)
            nc.sync.dma_start(idx_t[:], idxs[:])

            pat0_ap = cst_t[:, PAT0_OFF:PAT0_OFF + 12 * NBIN].rearrange(
                "p (c b) -> p c b", b=NBIN)
            pat1_ap = cst_t[:, PAT1_OFF:PAT1_OFF + 7 * NBIN].rearrange(
                "p (c b) -> p c b", b=NBIN)
            wcol0_ap = cst_t[:, WCOL0_OFF:WCOL0_OFF + NROI_CORE * 12].rearrange(
                "p (r c) -> p r c", c=12)
            wcol1_ap = cst_t[:, WCOL1_OFF:WCOL1_OFF + NROI_CORE * 7].rearrange(
                "p (r c) -> p r c", c=7)

            for grp in range(NGRP):
                # L2/L3 block gathers + lhsT block loads
                if grp % (BLK2 // GRP) == 0:
                    blk = grp // (BLK2 // GRP)
                    gt2 = g2p.tile([128, BLK2 * L2['NCH'], C], BF, tag="g2")
                    io = IDX2_OFF + blk * IC2
                    nc.gpsimd.dma_gather(
                        gt2[:], f2[:], idx_t[:, io:io + IC2],
                        BLK2 * L2['NJ'], BLK2 * L2['NJ'], C)
                    lt2_t = ltp.tile([128, BLK2, L2['NCH'] * NBIN], BF, tag="lt2")
                    nc.sync.dma_start(
                        lt2_t[:],
                        lt2[blk * BLK2:(blk + 1) * BLK2].rearrange("r p m -> p r m"))
                if grp % (BLK3 // GRP) == 0:
                    blk = grp // (BLK3 // GRP)
                    gt3 = g3p.tile([128, BLK3, C], BF, tag="g3")
                    io = IDX3_OFF + blk * IC3
                    nc.gpsimd.dma_gather(
                        gt3[:], f3[:], idx_t[:, io:io + IC3],
                        BLK3 * L3['NJ'], BLK3 * L3['NJ'], C)
                    lt3_t = ltp.tile([128, BLK3, NBIN], BF, tag="lt3")
                    nc.sync.dma_start(
                        lt3_t[:],
                        lt3[blk * BLK3:(blk + 1) * BLK3].rearrange("r p m -> p r m"))

                # group gathers: L0 (3px tri elems), L1 (1px)
                gt0 = g0p.tile([128, GRP * L0['NCH'], 3 * C], BF, tag="g0")
                io = IDX0_OFF + grp * IC0
                nc.gpsimd.dma_gather(
                    gt0[:], f0_view, idx_t[:, io:io + IC0],
                    GRP * L0['NJ'], GRP * L0['NJ'], 3 * C, elem_step=2 * C)
                gt1 = g1p.tile([128, GRP * L1['NCH'], C], BF, tag="g1")
                io = IDX1_OFF + grp * IC1
                hc = IC1 // GRP
                for r2 in range(GRP):
                    # SWDGE ring caps a call at 1024 descriptors: one call per ROI
                    nc.gpsimd.dma_gather(
                        gt1[:, r2 * L1['NCH']:(r2 + 1) * L1['NCH'], :], f1[:],
                        idx_t[:, io + r2 * hc:io + (r2 + 1) * hc],
                        L1['NJ'], L1['NJ'], C)

                ev = evp.tile([NBIN, GRP, C], BF, tag="ev")
                for r2 in range(GRP):
                    roi = grp * GRP + r2
                    # build W tiles: pattern * per-partition scalar (broadcast)
                    w0 = wp.tile([128, 12, NBIN], BF, tag="w0")
                    nc.vector.tensor_tensor(
                        w0[:], pat0_ap,
                        wcol0_ap[:, roi, :].unsqueeze(2).to_broadcast([128, 12, NBIN]),
                        mybir.AluOpType.mult)
                    w1 = wp.tile([128, 7, NBIN], BF, tag="w1")
                    nc.vector.tensor_tensor(
                        w1[:], pat1_ap,
                        wcol1_ap[:, roi, :].unsqueeze(2).to_broadcast([128, 7, NBIN]),
                        mybir.AluOpType.mult)

                    acc = accp.tile([NBIN, C], F32)
                    n_mm = 12 + 7 + 3 + 1
                    mi = 0
                    for c in range(L0['NCH']):
                        for sl3 in range(3):
                            nc.tensor.matmul(
                                acc[:], w0[:, c * 3 + sl3, :],
                                gt0[:, r2 * L0['NCH'] + c, sl3 * C:(sl3 + 1) * C],
                                start=(mi == 0), stop=(mi == n_mm - 1))
                            mi += 1
                    for c in range(L1['NCH']):
                        nc.tensor.matmul(
                            acc[:], w1[:, c, :], gt1[:, r2 * L1['NCH'] + c, :],
                            start=(mi == 0), stop=(mi == n_mm - 1))
                        mi += 1
                    rb2 = roi % BLK2
                    lt2_ap = lt2_t[:, rb2, :].rearrange("p (c b) -> p c b", b=NBIN)
                    for c in range(L2['NCH']):
                        nc.tensor.matmul(
                            acc[:], lt2_ap[:, c, :],
                            gt2[:, rb2 * L2['NCH'] + c, :],
                            start=(mi == 0), stop=(mi == n_mm - 1))
                        mi += 1
                    rb3 = roi % BLK3
                    nc.tensor.matmul(
                        acc[:], lt3_t[:, rb3, :], gt3[:, rb3, :],
                        start=(mi == 0), stop=(mi == n_mm - 1))
                    mi += 1

                    nc.scalar.copy(ev[:, r2, :], acc[:])

                dst = out[grp * GRP:(grp + 1) * GRP].rearrange("r b c -> b r c")
                nc.sync.dma_start(dst, ev[:])
    nc.finalize()
    return nc


def kernel(x0, x1, x2, x3, boxes):
    from concourse.bass_utils import run_bass_kernel_spmd
    in_maps = _host_prepare(x0, x1, x2, x3, boxes)
    if 'nc' not in _MODULE_CACHE:
        _MODULE_CACHE['nc'] = _build_module()
    nc = _MODULE_CACHE['nc']
    res = run_bass_kernel_spmd(nc, in_maps, list(range(8)))
    globals()['_LAST_RESULTS'] = res
    outs = [np.asarray(res.results[k]["out"]) for k in range(8)]
    full = np.concatenate(outs, axis=0)            # [1024, 49, 256] bf16
    full = full.astype(np.float32).transpose(0, 2, 1)
    return np.ascontiguousarray(full.reshape(1024, C, POOLED, POOLED))


# revision 8
# speedup vs baseline: 2.0907x; 1.8992x over previous
"""Multi-level ROI Align (FPN pooler, 4 levels summed) on 8 Trainium2 cores.

Strategy: shard ROIs across cores (core k: batch k//4, 128 ROIs). Gather
indices and bilinear weights are computed on host from `boxes`; the device
kernel does HBM pixel gathers (bf16) + weighted scatter-reduction into 7x7
bins via PSUM-accumulating bf16 matmuls.

Per ROI, per level:  out[bin, c] = sum_j W[j, bin] * G[j, c]
where G rows are gathered bf16 pixel vectors (C=256) and W is either
fixed_pattern * per-partition scalar built on DVE (L0/L1, one-hot j->bin)
or host-baked dense bf16 lhsT (L2/L3 region mode).

L0 uses 3-px elements addressed at even-pixel granularity (idx = flat//2)
to fit the int16 index range (200*200 = 40000 > 32767).

Output is accumulated in fp32 PSUM, evacuated as bf16 [49, C] per ROI and
DMA'd straight to HBM; the host does the final [49,C] -> [C,7,7] transpose.
"""
import sys
import numpy as np
import ml_dtypes

sys.path.insert(0, '/opt/trn_rl_repo')

BF16 = ml_dtypes.bfloat16

POOLED = 7
SAMP = 2
NBIN = 49
C = 256
IMG = 800.0

# per level: H, W, scale, mode ('tri' 3px elems idx=flat//2 | 'px' 1px | 'reg' region px)
L0 = dict(H=200, W=200, scale=0.25, mode='tri', NJ=512, REAL=392, NCH=4)
L1 = dict(H=100, W=100, scale=0.125, mode='px', NJ=896, REAL=784, NCH=7)
L2 = dict(H=50, W=50, scale=0.0625, mode='reg', NJ=384, REAL=324, NCH=3, WREG=18)
L3 = dict(H=25, W=25, scale=0.03125, mode='reg', NJ=128, REAL=100, NCH=1, WREG=10)
LEVELS = [L0, L1, L2, L3]

NROI_CORE = 128     # ROIs per core
NGRP = 64           # groups of 2 ROIs
GRP = 2
BLK2 = 2            # ROIs per L2 gather call (HW SWDGE ring caps a call at 1024 descs)
BLK3 = 8            # ROIs per L3 gather call

# padded flat pixel counts of the feature buffers
F0_ROWS = 40004     # covers 3-px elem overrun
F1_ROWS = 10000
F2_ROWS = 3400      # covers region overrun (y,x up to 66)
F3_ROWS = 900       # covers region overrun (y,x up to 33)

# const bf16 column layout (per partition)
PAT0_OFF = 0                                  # [12, 49] pattern expanded per slot
PAT1_OFF = PAT0_OFF + 12 * NBIN               # [7, 49]
WCOL0_OFF = PAT1_OFF + 7 * NBIN               # [128 roi * 12]
WCOL1_OFF = WCOL0_OFF + NROI_CORE * 12        # [128 roi * 7]
CST_COLS = WCOL1_OFF + NROI_CORE * 7

# idx int16 column layout (per partition)
IC0 = GRP * L0['NJ'] // 16                    # 64 cols per 2-ROI group
IC1 = GRP * L1['NJ'] // 16                    # 112
IC2 = BLK2 * L2['NJ'] // 16                   # 192 cols per 8-ROI block
IC3 = BLK3 * L3['NJ'] // 16                   # 128 cols per 16-ROI block
IDX0_OFF = 0
IDX1_OFF = IDX0_OFF + NGRP * IC0
IDX2_OFF = IDX1_OFF + NGRP * IC1
IDX3_OFF = IDX2_OFF + (NROI_CORE // BLK2) * IC2
IDX_COLS = IDX3_OFF + (NROI_CORE // BLK3) * IC3

_MODULE_CACHE = {}


def _sample_meta(boxes_b, H, W, scale):
    """Per-ROI sample geometry in fp32, matching reference op order.
    boxes_b: [N, 4] fp32. Returns dict of [N,7,2] arrays."""
    f = np.float32
    b = boxes_b.astype(np.float32)
    x1 = b[:, 0] * f(scale)
    y1 = b[:, 1] * f(scale)
    x2 = b[:, 2] * f(scale)
    y2 = b[:, 3] * f(scale)
    rw = np.maximum(x2 - x1, f(1.0))
    rh = np.maximum(y2 - y1, f(1.0))
    bw = rw / f(POOLED)
    bh = rh / f(POOLED)
    g = (np.arange(POOLED, dtype=np.float32)[:, None]
         + (np.arange(SAMP, dtype=np.float32)[None, :] + f(0.5)) / f(SAMP))
    y = y1[:, None, None] + g[None] * bh[:, None, None]   # [N,7,2]
    x = x1[:, None, None] + g[None] * bw[:, None, None]
    masky = ((y >= f(-1.0)) & (y <= f(H))).astype(np.float32)
    maskx = ((x >= f(-1.0)) & (x <= f(W))).astype(np.float32)
    yc = np.clip(y, f(0.0), f(H - 1))
    xc = np.clip(x, f(0.0), f(W - 1))
    yl = np.floor(yc).astype(np.int64)
    xl = np.floor(xc).astype(np.int64)
    yh = np.minimum(yl + 1, H - 1)
    xh = np.minimum(xl + 1, W - 1)
    ly = (yc - yl.astype(np.float32)).astype(np.float32)
    lx = (xc - xl.astype(np.float32)).astype(np.float32)
    hy = (f(1.0) - ly).astype(np.float32)
    hx = (f(1.0) - lx).astype(np.float32)
    return dict(yl=yl, yh=yh, xl=xl, xh=xh, ly=ly, lx=lx, hy=hy, hx=hx,
                masky=masky, maskx=maskx, x=x, y=y)


def _build_tri(meta, lv):
    """L0: j = (rs, py, sy, px, sx) -> 392 3-px elems, 3 slot weights.
    Returns idx [N, NJ] int64, w [N, NJ, 3] fp32."""
    N = meta['yl'].shape[0]
    W = lv['W']
    NJ, REAL = lv['NJ'], lv['REAL']
    rows = np.stack([meta['yl'], meta['yh']], axis=1)          # [N,2,7,2] (rs)
    wys = np.stack([meta['hy'], meta['ly']], axis=1)           # [N,2,7,2]
    m = (meta['masky'][:, :, :, None, None] * meta['maskx'][:, None, None, :, :])  # [N,7,2,7,2]
    row = np.broadcast_to(rows[:, :, :, :, None, None], (N, 2, 7, 2, 7, 2))
    wy = np.broadcast_to(wys[:, :, :, :, None, None], (N, 2, 7, 2, 7, 2)).astype(np.float32)
    xl = np.broadcast_to(meta['xl'][:, None, None, None, :, :], (N, 2, 7, 2, 7, 2))
    hx = np.broadcast_to(meta['hx'][:, None, None, None, :, :], (N, 2, 7, 2, 7, 2)).astype(np.float32)
    lx = np.broadcast_to(meta['lx'][:, None, None, None, :, :], (N, 2, 7, 2, 7, 2)).astype(np.float32)
    mm = np.broadcast_to(m[:, None], (N, 2, 7, 2, 7, 2)).astype(np.float32)
    flat = row * W + xl
    idx = (flat >> 1).reshape(N, REAL)
    r = (flat & 1).astype(np.float32).reshape(N, REAL)
    wl = (wy * hx * mm * np.float32(0.25)).reshape(N, REAL)
    wh = (wy * lx * mm * np.float32(0.25)).reshape(N, REAL)
    w = np.zeros((N, NJ, 3), np.float32)
    w[:, :REAL, 0] = wl * (1 - r)
    w[:, :REAL, 1] = wl * r + wh * (1 - r)
    w[:, :REAL, 2] = wh * r
    idx_full = np.zeros((N, NJ), np.int64)
    idx_full[:, :REAL] = idx
    return idx_full, w


def _build_px(meta, lv):
    """L1: j = (rs, cs, py, sy, px, sx) -> 784 1-px corner gathers.
    Returns idx [N, NJ] int64, w [N, NJ] fp32."""
    N = meta['yl'].shape[0]
    W = lv['W']
    NJ, REAL = lv['NJ'], lv['REAL']
    rows = np.stack([meta['yl'], meta['yh']], axis=1)   # [N,2(rs),7,2]
    wys = np.stack([meta['hy'], meta['ly']], axis=1)
    cols = np.stack([meta['xl'], meta['xh']], axis=1)   # [N,2(cs),7,2]
    wxs = np.stack([meta['hx'], meta['lx']], axis=1)
    m = (meta['masky'][:, :, :, None, None] * meta['maskx'][:, None, None, :, :])
    row = np.broadcast_to(rows[:, :, None, :, :, None, None], (N, 2, 2, 7, 2, 7, 2))
    wy = np.broadcast_to(wys[:, :, None, :, :, None, None], (N, 2, 2, 7, 2, 7, 2)).astype(np.float32)
    col = np.broadcast_to(cols[:, None, :, None, None, :, :], (N, 2, 2, 7, 2, 7, 2))
    wx = np.broadcast_to(wxs[:, None, :, None, None, :, :], (N, 2, 2, 7, 2, 7, 2)).astype(np.float32)
    mm = np.broadcast_to(m[:, None, None], (N, 2, 2, 7, 2, 7, 2)).astype(np.float32)
    idx = (row * W + col).reshape(N, REAL)
    w = (wy * wx * mm * np.float32(0.25)).reshape(N, REAL)
    idx_full = np.zeros((N, NJ), np.int64)
    w_full = np.zeros((N, NJ), np.float32)
    idx_full[:, :REAL] = idx
    w_full[:, :REAL] = w
    return idx_full, w_full


def _build_reg(meta, lv):
    """L2/L3: bounding-region pixels + separable host-baked weights.
    Returns idx [N, NJ] int64, lhsT [N, NJ, 49] fp32."""
    N = meta['yl'].shape[0]
    H, W, WREG = lv['H'], lv['W'], lv['WREG']
    NJ, REAL = lv['NJ'], lv['REAL']
    f = np.float32
    y_base = np.floor(np.clip(meta['y'].reshape(N, -1).min(1), 0.0, H - 1)).astype(np.int64)
    x_base = np.floor(np.clip(meta['x'].reshape(N, -1).min(1), 0.0, W - 1)).astype(np.int64)
    WY = np.zeros((N, WREG, POOLED), np.float32)
    WX = np.zeros((N, WREG, POOLED), np.float32)
    ridx = np.arange(N)[:, None, None]
    pidx = np.broadcast_to(np.arange(POOLED)[None, :, None], (N, POOLED, SAMP))
    np.add.at(WY, (ridx, meta['yl'] - y_base[:, None, None], pidx),
              (f(0.5) * meta['hy'] * meta['masky']).astype(np.float32))
    np.add.at(WY, (ridx, meta['yh'] - y_base[:, None, None], pidx),
              (f(0.5) * meta['ly'] * meta['masky']).astype(np.float32))
    np.add.at(WX, (ridx, meta['xl'] - x_base[:, None, None], pidx),
              (f(0.5) * meta['hx'] * meta['maskx']).astype(np.float32))
    np.add.at(WX, (ridx, meta['xh'] - x_base[:, None, None], pidx),
              (f(0.5) * meta['lx'] * meta['maskx']).astype(np.float32))
    lhsT = np.einsum('nap,nbq->nabpq', WY, WX).reshape(N, REAL, NBIN)
    dy = np.arange(WREG)
    idx = ((y_base[:, None, None] + dy[None, :, None]) * W
           + x_base[:, None, None] + dy[None, None, :]).reshape(N, REAL)
    idx_full = np.zeros((N, NJ), np.int64)
    lhsT_full = np.zeros((N, NJ, NBIN), np.float32)
    idx_full[:, :REAL] = idx
    lhsT_full[:, :REAL] = lhsT
    return idx_full, lhsT_full


def _pack_idx(jlists):
    """Pack concatenated idx list [..., n] -> [..., 128, n//16]
    int16 wrapped in 16 partitions, replicated 8x."""
    jl = np.asarray(jlists)
    n = jl.shape[-1]
    arr = jl.reshape(*jl.shape[:-1], n // 16, 16)   # [..., col, p]
    arr = np.swapaxes(arr, -1, -2)                  # [..., p(16), col]
    arr = np.broadcast_to(arr[..., None, :, :],
                          (*jl.shape[:-1], 8, 16, n // 16))
    return arr.reshape(*jl.shape[:-1], 128, n // 16).astype(np.int16)


def _bin_pattern(mode, NCH, REAL, nslot):
    """Fixed j->bin one-hot pattern [128, NCH*nslot, 49] (expanded per slot)."""
    NJ = NCH * 128
    j = np.arange(NJ)
    # j = ((((rs*7+py)*2+sy)*7+px)*2+sx)  (same py/px decode for tri & px)
    px = (j // 2) % 7
    py = (j // (2 * 7 * 2)) % 7
    bins = py * 7 + px
    pat = np.zeros((NJ, NBIN), np.float32)
    valid = j < REAL
    pat[np.arange(NJ)[valid], bins[valid]] = 1.0
    pat = pat.reshape(NCH, 128, NBIN).transpose(1, 0, 2)          # [128, NCH, 49]
    pat = np.repeat(pat[:, :, None, :], nslot, axis=2)            # [128, NCH, nslot, 49]
    return pat.reshape(128, NCH * nslot, NBIN)


def _host_prepare(x0, x1, x2, x3, boxes):
    """Build all per-core input tensors. Returns list of 8 dicts."""
    B = boxes.shape[0]
    feats = []
    for arr, lv, rows in ((x0, L0, F0_ROWS), (x1, L1, F1_ROWS),
                          (x2, L2, F2_ROWS), (x3, L3, F3_ROWS)):
        f = np.zeros((B, rows, C), BF16)
        hw = lv['H'] * lv['W']
        f[:, :hw] = np.ascontiguousarray(
            np.transpose(np.asarray(arr, np.float32), (0, 2, 3, 1))).reshape(B, hw, C).astype(BF16)
        feats.append(f)

    per_batch = []
    for b in range(B):
        bb = np.asarray(boxes[b], np.float32)
        m0 = _sample_meta(bb, L0['H'], L0['W'], L0['scale'])
        m1 = _sample_meta(bb, L1['H'], L1['W'], L1['scale'])
        m2 = _sample_meta(bb, L2['H'], L2['W'], L2['scale'])
        m3 = _sample_meta(bb, L3['H'], L3['W'], L3['scale'])
        idx0, w0 = _build_tri(m0, L0)
        idx1, w1 = _build_px(m1, L1)
        idx2, lt2 = _build_reg(m2, L2)
        idx3, lt3 = _build_reg(m3, L3)
        per_batch.append((idx0, w0, idx1, w1, idx2, lt2, idx3, lt3))

    pat0 = _bin_pattern('tri', L0['NCH'], L0['REAL'], 3)   # [128, 12, 49]
    pat1 = _bin_pattern('px', L1['NCH'], L1['REAL'], 1)    # [128, 7, 49]

    in_maps = []
    for k in range(8):
        b = k // 4
        s = (k % 4) * NROI_CORE
        idx0, w0, idx1, w1, idx2, lt2, idx3, lt3 = per_batch[b]
        sl = slice(s, s + NROI_CORE)

        cst = np.zeros((128, CST_COLS), BF16)
        cst[:, PAT0_OFF:PAT0_OFF + 12 * NBIN] = pat0.reshape(128, -1).astype(BF16)
        cst[:, PAT1_OFF:PAT1_OFF + 7 * NBIN] = pat1.reshape(128, -1).astype(BF16)
        # wcol0 [128, roi*12]: col roi*12 + c*3 + slot = w0[roi, c*128+p, slot]
        wc0 = w0[sl].reshape(NROI_CORE, L0['NCH'], 128, 3)   # [roi,c,p,s]
        cst[:, WCOL0_OFF:WCOL0_OFF + NROI_CORE * 12] = (
            wc0.transpose(2, 0, 1, 3).reshape(128, -1).astype(BF16))
        wc1 = w1[sl].reshape(NROI_CORE, L1['NCH'], 128)      # [roi,c,p]
        cst[:, WCOL1_OFF:WCOL1_OFF + NROI_CORE * 7] = (
            wc1.transpose(2, 0, 1).reshape(128, -1).astype(BF16))

        idxs = np.zeros((128, IDX_COLS), np.int16)
        idxs[:, IDX0_OFF:IDX0_OFF + NGRP * IC0] = _pack_idx(
            idx0[sl].reshape(NGRP, GRP * L0['NJ'])).transpose(1, 0, 2).reshape(128, -1)
        idxs[:, IDX1_OFF:IDX1_OFF + NGRP * IC1] = _pack_idx(
            idx1[sl].reshape(NGRP, GRP * L1['NJ'])).transpose(1, 0, 2).reshape(128, -1)
        idxs[:, IDX2_OFF:IDX2_OFF + (NROI_CORE // BLK2) * IC2] = _pack_idx(
            idx2[sl].reshape(NROI_CORE // BLK2, BLK2 * L2['NJ'])).transpose(1, 0, 2).reshape(128, -1)
        idxs[:, IDX3_OFF:IDX3_OFF + (NROI_CORE // BLK3) * IC3] = _pack_idx(
            idx3[sl].reshape(NROI_CORE // BLK3, BLK3 * L3['NJ'])).transpose(1, 0, 2).reshape(128, -1)

        # lhsT k-major bf16: lt2 [roi, NJ(=3*128), 49] -> [roi, 128, 3*49]
        lt2k = np.ascontiguousarray(
            lt2[sl].reshape(NROI_CORE, L2['NCH'], 128, NBIN).transpose(0, 2, 1, 3)
        ).reshape(NROI_CORE, 128, L2['NCH'] * NBIN).astype(BF16)
        lt3k = np.ascontiguousarray(lt3[sl].reshape(NROI_CORE, 128, NBIN)).astype(BF16)

        in_maps.append({
            "f0": feats[0][b], "f1": feats[1][b],
            "f2": feats[2][b], "f3": feats[3][b],
            "cst": cst, "idxs": idxs, "lt2": lt2k, "lt3": lt3k,
        })
    return in_maps


def _build_module():
    from concourse import bacc, tile
    from concourse.bass import mybir
    import concourse.bass as bass_mod

    F32 = mybir.dt.float32
    BF = mybir.dt.bfloat16
    I16 = mybir.dt.int16
    AP = bass_mod.AP

    nc = bacc.Bacc(None, target_bir_lowering=False, num_swdge_queues=4)
    f0 = nc.dram_tensor("f0", [F0_ROWS, C], BF, kind="ExternalInput")
    f1 = nc.dram_tensor("f1", [F1_ROWS, C], BF, kind="ExternalInput")
    f2 = nc.dram_tensor("f2", [F2_ROWS, C], BF, kind="ExternalInput")
    f3 = nc.dram_tensor("f3", [F3_ROWS, C], BF, kind="ExternalInput")
    cst = nc.dram_tensor("cst", [128, CST_COLS], BF, kind="ExternalInput")
    idxs = nc.dram_tensor("idxs", [128, IDX_COLS], I16, kind="ExternalInput")
    lt2 = nc.dram_tensor("lt2", [NROI_CORE, 128, L2['NCH'] * NBIN], BF, kind="ExternalInput")
    lt3 = nc.dram_tensor("lt3", [NROI_CORE, 128, NBIN], BF, kind="ExternalInput")
    out = nc.dram_tensor("out", [NROI_CORE, NBIN, C], BF, kind="ExternalOutput")

    # overlapping 3-px elem view of f0: stride 2px, width 3px
    f0_view = AP(f0, 0, [[2 * C, F0_ROWS // 2 - 1], [1, 3 * C]])

    with tile.TileContext(nc) as tc:
        with (
            tc.tile_pool(name="const", bufs=1) as constp,
            tc.tile_pool(name="g0p", bufs=2) as g0p,
            tc.tile_pool(name="g1p", bufs=2) as g1p,
            tc.tile_pool(name="g2p", bufs=2) as g2p,
            tc.tile_pool(name="g3p", bufs=2) as g3p,
            tc.tile_pool(name="ltp", bufs=2) as ltp,
            tc.tile_pool(name="wp", bufs=4) as wp,
            tc.tile_pool(name="accp", bufs=4, space="PSUM") as accp,
            tc.tile_pool(name="evp", bufs=3) as evp,
        ):
            cst_t = constp.tile([128, CST_COLS], BF)
            nc.sync.dma_start(cst_t[:], cst[:])
            idx_t = constp.tile([128, IDX_COLS], I16)
            nc.sync.dma_start(idx_t[:], idxs[:])

            pat0_ap = cst_t[:, PAT0_OFF:PAT0_OFF + 12 * NBIN].rearrange(
                "p (c b) -> p c b", b=NBIN)
            pat1_ap = cst_t[:, PAT1_OFF:PAT1_OFF + 7 * NBIN].rearrange(
                "p (c b) -> p c b", b=NBIN)
            wcol0_ap = cst_t[:, WCOL0_OFF:WCOL0_OFF + NROI_CORE * 12].rearrange(
                "p (r c) -> p r c", c=12)
            wcol1_ap = cst_t[:, WCOL1_OFF:WCOL1_OFF + NROI_CORE * 7].rearrange(
                "p (r c) -> p r c", c=7)

            for grp in range(NGRP):
                # L2/L3 block gathers + lhsT block loads
                if grp % (BLK2 // GRP) == 0:
                    blk = grp // (BLK2 // GRP)
                    gt2 = g2p.tile([128, BLK2 * L2['NCH'], C], BF, tag="g2")
                    io = IDX2_OFF + blk * IC2
                    nc.gpsimd.dma_gather(
                        gt2[:], f2[:], idx_t[:, io:io + IC2],
                        BLK2 * L2['NJ'], BLK2 * L2['NJ'], C, queue_num=2)
                    lt2_t = ltp.tile([128, BLK2, L2['NCH'] * NBIN], BF, tag="lt2")
                    nc.sync.dma_start(
                        lt2_t[:],
                        lt2[blk * BLK2:(blk + 1) * BLK2].rearrange("r p m -> p r m"))
                if grp % (BLK3 // GRP) == 0:
                    blk = grp // (BLK3 // GRP)
                    gt3 = g3p.tile([128, BLK3, C], BF, tag="g3")
                    io = IDX3_OFF + blk * IC3
                    nc.gpsimd.dma_gather(
                        gt3[:], f3[:], idx_t[:, io:io + IC3],
                        BLK3 * L3['NJ'], BLK3 * L3['NJ'], C, queue_num=3)
                    lt3_t = ltp.tile([128, BLK3, NBIN], BF, tag="lt3")
                    nc.sync.dma_start(
                        lt3_t[:],
                        lt3[blk * BLK3:(blk + 1) * BLK3].rearrange("r p m -> p r m"))

                # group gathers: L0 (3px tri elems), L1 (1px)
                gt0 = g0p.tile([128, GRP * L0['NCH'], 3 * C], BF, tag="g0")
                io = IDX0_OFF + grp * IC0
                nc.gpsimd.dma_gather(
                    gt0[:], f0_view, idx_t[:, io:io + IC0],
                    GRP * L0['NJ'], GRP * L0['NJ'], 3 * C, elem_step=2 * C,
                    queue_num=0)
                gt1 = g1p.tile([128, GRP * L1['NCH'], C], BF, tag="g1")
                io = IDX1_OFF + grp * IC1
                hc = IC1 // GRP
                for r2 in range(GRP):
                    # SWDGE ring caps a call at 1024 descriptors: one call per ROI
                    nc.gpsimd.dma_gather(
                        gt1[:, r2 * L1['NCH']:(r2 + 1) * L1['NCH'], :], f1[:],
                        idx_t[:, io + r2 * hc:io + (r2 + 1) * hc],
                        L1['NJ'], L1['NJ'], C, queue_num=1 + (r2 + grp) % 2 * 2)

                ev = evp.tile([NBIN, GRP, C], BF, tag="ev")
                for r2 in range(GRP):
                    roi = grp * GRP + r2
                    # build W tiles: pattern * per-partition scalar (broadcast)
                    w0 = wp.tile([128, 12, NBIN], BF, tag="w0")
                    nc.vector.tensor_tensor(
                        w0[:], pat0_ap,
                        wcol0_ap[:, roi, :].unsqueeze(2).to_broadcast([128, 12, NBIN]),
                        mybir.AluOpType.mult)
                    w1 = wp.tile([128, 7, NBIN], BF, tag="w1")
                    nc.vector.tensor_tensor(
                        w1[:], pat1_ap,
                        wcol1_ap[:, roi, :].unsqueeze(2).to_broadcast([128, 7, NBIN]),
                        mybir.AluOpType.mult)

                    acc = accp.tile([NBIN, C], F32)
                    n_mm = 12 + 7 + 3 + 1
                    mi = 0
                    for c in range(L0['NCH']):
                        for sl3 in range(3):
                            nc.tensor.matmul(
                                acc[:], w0[:, c * 3 + sl3, :],
                                gt0[:, r2 * L0['NCH'] + c, sl3 * C:(sl3 + 1) * C],
                                start=(mi == 0), stop=(mi == n_mm - 1))
                            mi += 1
                    for c in range(L1['NCH']):
                        nc.tensor.matmul(
                            acc[:], w1[:, c, :], gt1[:, r2 * L1['NCH'] + c, :],
                            start=(mi == 0), stop=(mi == n_mm - 1))
                        mi += 1
                    rb2 = roi % BLK2
                    lt2_ap = lt2_t[:, rb2, :].rearrange("p (c b) -> p c b", b=NBIN)
                    for c in range(L2['NCH']):
                        nc.tensor.matmul(
                            acc[:], lt2_ap[:, c, :],
                            gt2[:, rb2 * L2['NCH'] + c, :],
                            start=(mi == 0), stop=(mi == n_mm - 1))
                        mi += 1
                    rb3 = roi % BLK3
                    nc.tensor.matmul(
                        acc[:], lt3_t[:, rb3, :], gt3[:, rb3, :],
                        start=(mi == 0), stop=(mi == n_mm - 1))
                    mi += 1

                    nc.scalar.copy(ev[:, r2, :], acc[:])

                dst = out[grp * GRP:(grp + 1) * GRP].rearrange("r b c -> b r c")
                nc.sync.dma_start(dst, ev[:])
    nc.finalize()
    return nc


def kernel(x0, x1, x2, x3, boxes):
    from concourse.bass_utils import run_bass_kernel_spmd
    in_maps = _host_prepare(x0, x1, x2, x3, boxes)
    if 'nc' not in _MODULE_CACHE:
        _MODULE_CACHE['nc'] = _build_module()
    nc = _MODULE_CACHE['nc']
    res = run_bass_kernel_spmd(nc, in_maps, list(range(8)))
    globals()['_LAST_RESULTS'] = res
    outs = [np.asarray(res.results[k]["out"]) for k in range(8)]
    full = np.concatenate(outs, axis=0)            # [1024, 49, 256] bf16
    full = full.astype(np.float32).transpose(0, 2, 1)
    return np.ascontiguousarray(full.reshape(1024, C, POOLED, POOLED))
